# revision 16
# baseline (speedup 1.0000x reference)
"""Conv1d(k=1) multi-head causal attention on 8 TRN2 NeuronCores.

Math (per batch b):
    q/k/v = x @ Wq/Wk/Wv^T          (kernel-size-1 conv == matmul)
    per head h: S = (q_h k_h^T)/8,  P = softmax(causal(S)),  ctx_h = P v_h
    out = concat_h(ctx_h) @ Wout^T + b_out

Sharding: 8 cores = 2 (batch) x 4 (head groups of 4 heads, 256 channels).
Each core computes q/k/v for its 4 heads, causal attention, and a partial
out-projection over its 256 context channels.  Host sums the 4 partials
per batch and adds the bias.

Device-side layouts (per core):
    xT   [d=1024, s=2048]  (x transposed on host)
    qT/kT: [c=256, s]  (c = head-major channels)  -> scores^T = kT_h^T-block
    v:   [s, c] natural, augmented with a ones column per head so the
         P^T-contraction matmul also produces the softmax denominator.
    scores^T [j, i] per (head, i-block): exp applied elementwise, then
         ctx^T[dh+1, i] accumulates over j via PE with v as stationary.
    Normalization: recip of denom row, partition-broadcast via a K=1
         matmul, multiply into ctxT.
    out partial [s, e] = ctxT^T @ woT.

All matmuls run in bf16 (PSUM accumulation stays fp32; softmax
denominators and normalization stay fp32).  The 1/sqrt(dh) scale is
folded into Wq on the host.
"""

from contextlib import ExitStack

import numpy as np

import concourse.bass as bass
import concourse.tile as tile
from concourse import bacc, mybir
from concourse import bass_utils

F32 = mybir.dt.float32
F32R = mybir.dt.float32r

B, S, D = 2, 2048, 1024
H_PER_CORE = 4          # heads per core
DH = 64                 # head dim
C = H_PER_CORE * DH     # 256 channels per core
E = 1024                # embed (out) dim
N_CORES = 8
NEG = -1.0e30

KC = D // 128           # 8 contraction chunks for projections
SB5 = S // 512          # 4 blocks of 512 along s
SB1 = S // 128          # 16 blocks of 128 along s


MM_DTYPE = mybir.dt.bfloat16


def build(mm_dtype=None):
    if mm_dtype is None:
        mm_dtype = MM_DTYPE
    nc = bacc.Bacc("TRN2", target_bir_lowering=False, debug=False,
                   enable_asserts=False, num_devices=N_CORES)
    xT = nc.dram_tensor("xT", (D, S), mm_dtype, kind="ExternalInput").ap()
    wq = nc.dram_tensor("wq", (D, C), mm_dtype, kind="ExternalInput").ap()
    wk = nc.dram_tensor("wk", (D, C), mm_dtype, kind="ExternalInput").ap()
    wv = nc.dram_tensor("wv", (D, C), mm_dtype, kind="ExternalInput").ap()
    wo = nc.dram_tensor("wo", (C, E), mm_dtype, kind="ExternalInput").ap()
    mask = nc.dram_tensor("mask", (128, 896), F32, kind="ExternalInput").ap()
    o = nc.dram_tensor("o", (S, E), F32, kind="ExternalOutput").ap()

    with tile.TileContext(nc) as tc, ExitStack() as ctx:
        const = ctx.enter_context(tc.tile_pool(name="const", bufs=1))
        persist = ctx.enter_context(tc.tile_pool(name="persist", bufs=1))
        work = ctx.enter_context(tc.tile_pool(name="work", bufs=1))
        psum = ctx.enter_context(tc.tile_pool(name="psum", bufs=1, space="PSUM"))

        mask_sb = const.tile([128, 896], F32)
        nc.sync.dma_start(mask_sb[:], mask)
        ones_sb = const.tile([1, DH], F32)
        nc.vector.memset(ones_sb[:], 1.0)

        w_r = {}
        for name, t in (("wq", wq), ("wk", wk), ("wv", wv)):
            w_r[name] = const.tile([128, KC, C], mm_dtype, name=f"w_{name}")
            nc.sync.dma_start(w_r[name][:], t.rearrange("(k p) c -> p k c", p=128))
        wo_r = const.tile([128, C // 128, E], mm_dtype)
        nc.sync.dma_start(wo_r[:], wo.rearrange("(k p) e -> p k e", p=128))

        qT = persist.tile([128, 2, S], mm_dtype)      # [c%128, c//128, s]
        kT = persist.tile([128, 2, S], mm_dtype)
        v_aug = persist.tile([128, SB1, H_PER_CORE, DH + 1], mm_dtype)
        ctxT = persist.tile([128, 2, S], mm_dtype)
        ones_col = const.tile([128, SB1 * H_PER_CORE], F32)
        nc.vector.memset(ones_col[:], 1.0)
        nc.vector.tensor_copy(
            v_aug[:, :, :, DH],
            ones_col[:].rearrange("p (a b) -> p a b", a=SB1))

        # ---- phase 1: load x, project k, v, q ----
        xr = []
        for k in range(KC):
            xk = work.tile([128, S], mm_dtype, tag=f"x{k}", name=f"x{k}")
            nc.sync.dma_start(xk[:], xT[k * 128:(k + 1) * 128, :])
            xr.append(xk)

        def proj_qk(w_name, dst, c2, s4):
            ps = psum.tile([128, 512], F32, tag="sc", bufs=2,
                           name=f"p_{w_name}{c2}{s4}")
            for k in range(KC):
                nc.tensor.matmul(
                    ps[:],
                    w_r[w_name][:, k, c2 * 128:(c2 + 1) * 128],
                    xr[k][:, s4 * 512:(s4 + 1) * 512],
                    start=(k == 0), stop=(k == KC - 1))
            nc.vector.tensor_copy(dst[:, c2, s4 * 512:(s4 + 1) * 512], ps[:])

        def proj_v(s1):
            ps = psum.tile([128, C], F32, tag="ctx", bufs=5, name=f"p_v{s1}")
            for k in range(KC):
                nc.tensor.matmul(
                    ps[:],
                    xr[k][:, s1 * 128:(s1 + 1) * 128],
                    w_r["wv"][:, k, :],
                    start=(k == 0), stop=(k == KC - 1))
            nc.vector.tensor_copy(
                v_aug[:, s1, :, 0:DH],
                ps[:].rearrange("p (h d) -> p h d", h=H_PER_CORE))

        for c2 in range(2):
            for s4 in range(SB5):
                proj_qk("wk", kT, c2, s4)
        for s1 in range(SB1):
            proj_v(s1)
        for s4 in range(SB5):          # q in s4-major order: i4=0 ready first
            for c2 in range(2):
                proj_qk("wq", qT, c2, s4)

        # ---- phase 2+3: attention (4 heads in flight) + fused out-proj
        def attend(i4):
            n_j = (i4 + 1) * 4
            pcs = [psum.tile([DH + 1, 512], F32, tag="ctx", bufs=5,
                             name=f"pc{i4}h{h}") for h in range(H_PER_CORE)]
            for jt in range(n_j):
                js0 = jt * 128 - i4 * 512
                pss, psts = [], []
                for h in range(H_PER_CORE):
                    hp, hc = (h % 2) * 64, h // 2
                    ps = psum.tile([128, 512], F32, tag="sc", bufs=2,
                                   name=f"ps{i4}h{h}j{jt}")
                    pss.append(ps)
                    nc.tensor.matmul(
                        ps[:],
                        kT[hp:hp + 64, hc, jt * 128:(jt + 1) * 128],
                        qT[hp:hp + 64, hc, i4 * 512:(i4 + 1) * 512],
                        start=True, stop=True)
                for h in range(H_PER_CORE):
                    if js0 >= 0:  # diagonal: mask covered cols only
                        w = js0 + 128
                        nc.vector.tensor_add(
                            pss[h][:, 0:w], pss[h][:, 0:w],
                            mask_sb[:, 384 - js0:384 - js0 + w])
                    pst = work.tile([128, 512], mm_dtype, tag="pst", bufs=8,
                                    name=f"pt{i4}h{h}j{jt}")
                    psts.append(pst)
                    nc.scalar.activation(
                        pst[:], pss[h][:], mybir.ActivationFunctionType.Exp)
                for h in range(H_PER_CORE):
                    nc.tensor.matmul(
                        pcs[h][:], v_aug[:, jt, h, :], psts[h][:],
                        start=(jt == 0), stop=(jt == n_j - 1),
                        skip_group_check=True)
            for h in range(H_PER_CORE):
                hp, hc = (h % 2) * 64, h // 2
                pc = pcs[h]
                recip = work.tile([1, 512], F32, tag="recip", bufs=2,
                                  name=f"rc{i4}h{h}")
                nc.vector.reciprocal(recip[:], pc[64:65, :])
                pb = psum.tile([64, 512], F32, tag="bc", bufs=1,
                               name=f"pb{i4}h{h}")
                nc.tensor.matmul(pb[:], ones_sb[:], recip[:],
                                 start=True, stop=True)
                bc = work.tile([64, 512], F32, tag="bcs", bufs=2,
                               name=f"bc{i4}h{h}")
                nc.vector.tensor_copy(bc[:], pb[:])
                nc.vector.tensor_mul(
                    ctxT[hp:hp + 64, hc, i4 * 512:(i4 + 1) * 512],
                    pc[0:64, :], bc[:])

        def out_proj(s1):
            out_sb = work.tile([128, E], F32, tag="osb", bufs=2,
                               name=f"os{s1}")
            for e2 in range(2):
                po = psum.tile([128, 512], F32, tag="ctx", bufs=5,
                               name=f"po{s1}e{e2}")
                for c2 in range(2):
                    nc.tensor.matmul(
                        po[:],
                        ctxT[:, c2, s1 * 128:(s1 + 1) * 128],
                        wo_r[:, c2, e2 * 512:(e2 + 1) * 512],
                        start=(c2 == 0), stop=(c2 == 1))
                nc.vector.tensor_copy(out_sb[:, e2 * 512:(e2 + 1) * 512],
                                      po[:])
            nc.sync.dma_start(o[s1 * 128:(s1 + 1) * 128, :], out_sb[:])

        for i4 in range(SB5):
            attend(i4)
            # out-proj for this i-window: dense PE filler overlapping the
            # next window's attention chains
            for s1 in range(i4 * 4, (i4 + 1) * 4):
                out_proj(s1)

    nc.compile()
    return nc


def make_mask():
    p = np.arange(128)[:, None]
    g = np.arange(896)[None, :]
    return np.where(g >= p + 384, 0.0, NEG).astype(np.float32)


def make_in_maps(x, wq, wk, wv, w_out, mm_dtype=None):
    """Per-core inputs. Core c: batch c//4, head-group c%4."""
    if mm_dtype is None:
        mm_dtype = MM_DTYPE
    if mm_dtype == mybir.dt.bfloat16:
        import ml_dtypes
        cast = lambda a: np.ascontiguousarray(a).astype(ml_dtypes.bfloat16)
    else:
        cast = lambda a: np.ascontiguousarray(a, dtype=np.float32)
    mask = make_mask()
    scale = DH ** (-0.5)
    in_maps = []
    for c in range(N_CORES):
        b, hg = c // 4, c % 4
        cs = slice(hg * C, (hg + 1) * C)
        in_maps.append({
            "xT": cast(x[b].T),
            "wq": cast((wq[cs, :, 0] * scale).T),
            "wk": cast(wk[cs, :, 0].T),
            "wv": cast(wv[cs, :, 0].T),
            "wo": cast(w_out[:, cs].T),
            "mask": mask,
        })
    return in_maps


_NC_CACHE = {}


def get_nc(mm_dtype=None):
    if mm_dtype is None:
        mm_dtype = MM_DTYPE
    key = str(mm_dtype)
    if key not in _NC_CACHE:
        _NC_CACHE[key] = build(mm_dtype)
    return _NC_CACHE[key]


def kernel(x, attn_mask, wq, wk, wv, w_out, b_out):
    x = np.asarray(x, dtype=np.float32)
    nc = get_nc()
    in_maps = make_in_maps(x, np.asarray(wq), np.asarray(wk),
                           np.asarray(wv), np.asarray(w_out))
    res = bass_utils.run_bass_kernel_spmd(nc, in_maps,
                                          core_ids=list(range(N_CORES)))
    out = np.zeros((B, S, E), dtype=np.float32)
    for c in range(N_CORES):
        out[c // 4] += res.results[c]["o"]
    out += np.asarray(b_out, dtype=np.float32)
    return out


# revision 17
# speedup vs baseline: 1.1276x; 1.1276x over previous
"""Conv1d(k=1) multi-head causal attention on 8 TRN2 NeuronCores.

Math (per batch b):
    q/k/v = x @ Wq/Wk/Wv^T          (kernel-size-1 conv == matmul)
    per head h: S = (q_h k_h^T)/8,  P = softmax(causal(S)),  ctx_h = P v_h
    out = concat_h(ctx_h) @ Wout^T + b_out

Sharding: 8 cores = 2 (batch) x 4 (head groups of 4 heads, 256 channels).
Each core computes q/k/v for its 4 heads, causal attention, and a partial
out-projection over its 256 context channels.  Host sums the 4 partials
per batch and adds the bias.

Device-side layouts (per core):
    xT   [d=1024, s=2048]  (x transposed on host)
    qT/kT: [c=256, s]  (c = head-major channels)  -> scores^T = kT_h^T-block
    v:   [s, c] natural, augmented with a ones column per head so the
         P^T-contraction matmul also produces the softmax denominator.
    scores^T [j, i] per (head, i-block): exp applied elementwise, then
         ctx^T[dh+1, i] accumulates over j via PE with v as stationary.
    Normalization: recip of denom row, partition-broadcast via a K=1
         matmul, multiply into ctxT.
    out partial [s, e] = ctxT^T @ woT.

All matmuls run in bf16 (PSUM accumulation stays fp32; softmax
denominators and normalization stay fp32).  The 1/sqrt(dh) scale is
folded into Wq on the host.
"""

from contextlib import ExitStack

import numpy as np

import concourse.bass as bass
import concourse.tile as tile
from concourse import bacc, mybir
from concourse import bass_utils

F32 = mybir.dt.float32
F32R = mybir.dt.float32r

B, S, D = 2, 2048, 1024
H_PER_CORE = 4          # heads per core
DH = 64                 # head dim
C = H_PER_CORE * DH     # 256 channels per core
E = 1024                # embed (out) dim
N_CORES = 8
NEG = -1.0e30

KC = D // 128           # 8 contraction chunks for projections
SB5 = S // 512          # 4 blocks of 512 along s
SB1 = S // 128          # 16 blocks of 128 along s


MM_DTYPE = mybir.dt.bfloat16


def build(mm_dtype=None):
    if mm_dtype is None:
        mm_dtype = MM_DTYPE
    nc = bacc.Bacc("TRN2", target_bir_lowering=False, debug=False,
                   enable_asserts=False, num_devices=N_CORES)
    xT = nc.dram_tensor("xT", (D, S), mm_dtype, kind="ExternalInput").ap()
    wq = nc.dram_tensor("wq", (D, C), mm_dtype, kind="ExternalInput").ap()
    wk = nc.dram_tensor("wk", (D, C), mm_dtype, kind="ExternalInput").ap()
    wv = nc.dram_tensor("wv", (D, C), mm_dtype, kind="ExternalInput").ap()
    wo = nc.dram_tensor("wo", (C, E), mm_dtype, kind="ExternalInput").ap()
    mask = nc.dram_tensor("mask", (128, 896), F32, kind="ExternalInput").ap()
    o = nc.dram_tensor("o", (S, E), F32, kind="ExternalOutput").ap()

    with tile.TileContext(nc) as tc, ExitStack() as ctx:
        const = ctx.enter_context(tc.tile_pool(name="const", bufs=1))
        persist = ctx.enter_context(tc.tile_pool(name="persist", bufs=1))
        work = ctx.enter_context(tc.tile_pool(name="work", bufs=1))
        psum = ctx.enter_context(tc.tile_pool(name="psum", bufs=1, space="PSUM"))

        mask_sb = const.tile([128, 896], F32)
        nc.sync.dma_start(mask_sb[:], mask)
        ones_sb = const.tile([1, DH], F32)
        nc.vector.memset(ones_sb[:], 1.0)

        w_r = {}
        for name, t in (("wq", wq), ("wk", wk), ("wv", wv)):
            w_r[name] = const.tile([128, KC, C], mm_dtype, name=f"w_{name}")
            nc.sync.dma_start(w_r[name][:], t.rearrange("(k p) c -> p k c", p=128))
        wo_r = const.tile([128, C // 128, E], mm_dtype)
        nc.sync.dma_start(wo_r[:], wo.rearrange("(k p) e -> p k e", p=128))

        qT = persist.tile([128, 2, S], mm_dtype)      # [c%128, c//128, s]
        kT = persist.tile([128, 2, S], mm_dtype)
        v_aug = persist.tile([128, SB1, H_PER_CORE, DH + 1], mm_dtype)
        ctxT = persist.tile([128, 2, S], mm_dtype)
        ones_col = const.tile([128, SB1 * H_PER_CORE], F32)
        nc.vector.memset(ones_col[:], 1.0)
        nc.vector.tensor_copy(
            v_aug[:, :, :, DH],
            ones_col[:].rearrange("p (a b) -> p a b", a=SB1))

        # ---- phase 1: load x, project k, v, q ----
        xr = []
        for k in range(KC):
            xk = work.tile([128, S], mm_dtype, tag=f"x{k}", name=f"x{k}")
            nc.sync.dma_start(xk[:], xT[k * 128:(k + 1) * 128, :])
            xr.append(xk)

        def proj_qk(w_name, dst, c2, s4):
            ps = psum.tile([128, 512], F32, tag="sc", bufs=3,
                           name=f"p_{w_name}{c2}{s4}")
            for k in range(KC):
                nc.tensor.matmul(
                    ps[:],
                    w_r[w_name][:, k, c2 * 128:(c2 + 1) * 128],
                    xr[k][:, s4 * 512:(s4 + 1) * 512],
                    start=(k == 0), stop=(k == KC - 1))
            nc.vector.tensor_copy(dst[:, c2, s4 * 512:(s4 + 1) * 512], ps[:])

        def proj_v(s1):
            ps = psum.tile([128, C], F32, tag="ctx", bufs=4, name=f"p_v{s1}")
            for k in range(KC):
                nc.tensor.matmul(
                    ps[:],
                    xr[k][:, s1 * 128:(s1 + 1) * 128],
                    w_r["wv"][:, k, :],
                    start=(k == 0), stop=(k == KC - 1))
            nc.vector.tensor_copy(
                v_aug[:, s1, :, 0:DH],
                ps[:].rearrange("p (h d) -> p h d", h=H_PER_CORE))

        for c2 in range(2):
            for s4 in range(SB5):
                proj_qk("wk", kT, c2, s4)
        for s1 in range(SB1):
            proj_v(s1)
        for s4 in range(SB5):          # q in s4-major order: i4=0 ready first
            for c2 in range(2):
                proj_qk("wq", qT, c2, s4)

        # ---- phase 2+3: attention (4 heads in flight) + fused out-proj
        def attend(i4):
            n_j = (i4 + 1) * 4
            pcs = [psum.tile([DH + 1, 512], F32, tag="ctx", bufs=4,
                             name=f"pc{i4}h{h}") for h in range(H_PER_CORE)]
            for jt in range(n_j):
                js0 = jt * 128 - i4 * 512
                pss, psts = [], []
                for h in range(H_PER_CORE):
                    hp, hc = (h % 2) * 64, h // 2
                    ps = psum.tile([128, 512], F32, tag="sc", bufs=3,
                                   name=f"ps{i4}h{h}j{jt}")
                    pss.append(ps)
                    nc.tensor.matmul(
                        ps[:],
                        kT[hp:hp + 64, hc, jt * 128:(jt + 1) * 128],
                        qT[hp:hp + 64, hc, i4 * 512:(i4 + 1) * 512],
                        start=True, stop=True)
                for h in range(H_PER_CORE):
                    if js0 >= 0:  # diagonal: mask covered cols only
                        w = js0 + 128
                        nc.vector.tensor_add(
                            pss[h][:, 0:w], pss[h][:, 0:w],
                            mask_sb[:, 384 - js0:384 - js0 + w])
                    pst = work.tile([128, 512], mm_dtype, tag="pst", bufs=8,
                                    name=f"pt{i4}h{h}j{jt}")
                    psts.append(pst)
                    nc.scalar.activation(
                        pst[:], pss[h][:], mybir.ActivationFunctionType.Exp)
                for h in range(H_PER_CORE):
                    nc.tensor.matmul(
                        pcs[h][:], v_aug[:, jt, h, :], psts[h][:],
                        start=(jt == 0), stop=(jt == n_j - 1),
                        skip_group_check=True)
            for h in range(H_PER_CORE):
                hp, hc = (h % 2) * 64, h // 2
                pc = pcs[h]
                recip = work.tile([1, 512], F32, tag="recip", bufs=2,
                                  name=f"rc{i4}h{h}")
                nc.vector.reciprocal(recip[:], pc[64:65, :])
                pb = psum.tile([64, 512], F32, tag="bc", bufs=1,
                               name=f"pb{i4}h{h}")
                nc.tensor.matmul(pb[:], ones_sb[:], recip[:],
                                 start=True, stop=True)
                bc = work.tile([64, 512], F32, tag="bcs", bufs=2,
                               name=f"bc{i4}h{h}")
                nc.vector.tensor_copy(bc[:], pb[:])
                nc.vector.tensor_mul(
                    ctxT[hp:hp + 64, hc, i4 * 512:(i4 + 1) * 512],
                    pc[0:64, :], bc[:])

        def out_proj(s1):
            out_sb = work.tile([128, E], F32, tag="osb", bufs=2,
                               name=f"os{s1}")
            for e2 in range(2):
                po = psum.tile([128, 512], F32, tag="bc", bufs=1,
                               name=f"po{s1}e{e2}")
                for c2 in range(2):
                    nc.tensor.matmul(
                        po[:],
                        ctxT[:, c2, s1 * 128:(s1 + 1) * 128],
                        wo_r[:, c2, e2 * 512:(e2 + 1) * 512],
                        start=(c2 == 0), stop=(c2 == 1))
                nc.vector.tensor_copy(out_sb[:, e2 * 512:(e2 + 1) * 512],
                                      po[:])
            nc.sync.dma_start(o[s1 * 128:(s1 + 1) * 128, :], out_sb[:])

        for i4 in range(SB5):
            attend(i4)
            # out-proj for this i-window: dense PE filler overlapping the
            # next window's attention chains
            for s1 in range(i4 * 4, (i4 + 1) * 4):
                out_proj(s1)

    nc.compile()
    return nc


def make_mask():
    p = np.arange(128)[:, None]
    g = np.arange(896)[None, :]
    return np.where(g >= p + 384, 0.0, NEG).astype(np.float32)


def make_in_maps(x, wq, wk, wv, w_out, mm_dtype=None):
    """Per-core inputs. Core c: batch c//4, head-group c%4."""
    if mm_dtype is None:
        mm_dtype = MM_DTYPE
    if mm_dtype == mybir.dt.bfloat16:
        import ml_dtypes
        cast = lambda a: np.ascontiguousarray(a).astype(ml_dtypes.bfloat16)
    else:
        cast = lambda a: np.ascontiguousarray(a, dtype=np.float32)
    mask = make_mask()
    scale = DH ** (-0.5)
    in_maps = []
    for c in range(N_CORES):
        b, hg = c // 4, c % 4
        cs = slice(hg * C, (hg + 1) * C)
        in_maps.append({
            "xT": cast(x[b].T),
            "wq": cast((wq[cs, :, 0] * scale).T),
            "wk": cast(wk[cs, :, 0].T),
            "wv": cast(wv[cs, :, 0].T),
            "wo": cast(w_out[:, cs].T),
            "mask": mask,
        })
    return in_maps


_NC_CACHE = {}


def get_nc(mm_dtype=None):
    if mm_dtype is None:
        mm_dtype = MM_DTYPE
    key = str(mm_dtype)
    if key not in _NC_CACHE:
        _NC_CACHE[key] = build(mm_dtype)
    return _NC_CACHE[key]


def kernel(x, attn_mask, wq, wk, wv, w_out, b_out):
    x = np.asarray(x, dtype=np.float32)
    nc = get_nc()
    in_maps = make_in_maps(x, np.asarray(wq), np.asarray(wk),
                           np.asarray(wv), np.asarray(w_out))
    res = bass_utils.run_bass_kernel_spmd(nc, in_maps,
                                          core_ids=list(range(N_CORES)))
    out = np.zeros((B, S, E), dtype=np.float32)
    for c in range(N_CORES):
        out[c // 4] += res.results[c]["o"]
    out += np.asarray(b_out, dtype=np.float32)
    return out
